# revision 9
# baseline (speedup 1.0000x reference)
"""Bahdanau additive attention kernel for Trainium2 (Bass/Tile).

Shapes (hardcoded from the problem spec):
  encoder_outputs [8, 256, 512] f32, decoder_outputs [8, 128, 512] f32,
  encoder_mask    [8, 256] bool,  W1/W2 [512, 512] f32,  V [512, 1] f32.

Sharding: data-parallel over batch B=8 across the 8 NeuronCores; the
W1/W2/V weights are replicated.  Each core computes one batch element:
  ew = enc @ W1; dw = dec @ W2
  score[t,s] = sum_h V[h] * tanh(ew[s,h] + dw[t,h]) + (m[s]-1)*1e9
  attn = softmax_s(score); ctx = attn @ enc

Engine split per core (the Td*Te*H = 16.7M element inner part):
  - DVE: builds X[h, (t,c,s)] = ew_T + dw_T[:, t] via per-partition-scalar
    adds (2x mode, 512 instrs of [128, 256])
  - ACT: tanh on [128, 8192] tiles (16 instrs -> amortizes the 222-cycle
    SBUF access bubble; ACT is the ~110us/core roofline of this problem)
  - PE : score accumulation via M=1 matmuls with V as stationary operand,
    mask add as a K=1 broadcast matmul, plus projections/transposes/context
"""

from contextlib import ExitStack

import numpy as np

import concourse.bass as bass
import concourse.tile as tile
from concourse import bacc, bass_utils, mybir
from concourse._compat import with_exitstack
from concourse.masks import make_identity

B, TD, TE, H = 8, 128, 256, 512
P = 128
HC = H // P  # 4 h-chunks
TG = 8  # t's per ACT group
NG = TD // TG
F32 = mybir.dt.float32
BF16 = mybir.dt.bfloat16


@with_exitstack
def _attention_kernel(ctx: ExitStack, tc: tile.TileContext, enc, dec, mask, w1, w2, v,
                      ctx_out, attn_out):
    nc = tc.nc
    add = mybir.AluOpType.add
    mult = mybir.AluOpType.mult

    singles = ctx.enter_context(tc.tile_pool(name="singles", bufs=1))
    xpool = ctx.enter_context(tc.tile_pool(name="xpool", bufs=2))
    ypool = ctx.enter_context(tc.tile_pool(name="ypool", bufs=2))
    psum = ctx.enter_context(tc.tile_pool(name="psum", bufs=2, space="PSUM"))
    score_pool = ctx.enter_context(tc.tile_pool(name="score", bufs=1, space="PSUM"))

    # ---------------- loads ----------------
    w1_sb = singles.tile([P, HC, H], F32)  # [p, hb, k] = W1[hb*128+p, k]
    nc.sync.dma_start(out=w1_sb, in_=w1.rearrange("(hb p) k -> p hb k", p=P))
    w2_sb = singles.tile([P, HC, H], F32)
    nc.sync.dma_start(out=w2_sb, in_=w2.rearrange("(hb p) k -> p hb k", p=P))
    enc_sb = singles.tile([P, 2, H], F32)  # [p, sb, h] = enc[sb*128+p, h]
    nc.sync.dma_start(out=enc_sb, in_=enc.rearrange("(sb p) h -> p sb h", p=P))
    dec_sb = singles.tile([P, H], F32)
    nc.sync.dma_start(out=dec_sb, in_=dec)
    v_sb32 = singles.tile([P, HC], F32)  # [p, c] = V[c*128+p, 0]
    nc.sync.dma_start(out=v_sb32, in_=v.rearrange("(c p) o -> p (c o)", p=P))
    mask_u8 = singles.tile([1, TE], mybir.dt.uint8)
    nc.sync.dma_start(out=mask_u8, in_=mask.rearrange("(o s) -> o s", o=1))

    # ---------------- constants ----------------
    ident = singles.tile([P, P], F32)
    make_identity(nc, ident)
    ones_row = singles.tile([1, P], BF16)
    nc.vector.memset(ones_row, 1.0)
    v_sb = singles.tile([P, HC], BF16)
    nc.vector.tensor_copy(out=v_sb, in_=v_sb32)
    # V32[:, c, j, :] is a [128, 32] one-hot-column stationary operand:
    # column j holds V chunk c, all other columns zero.  A matmul with it
    # accumulates V_c . Y into score row (strip_base + j) while adding zero
    # to the other 31 rows of the strip (PSUM out must be 32-row aligned).
    v32 = singles.tile([P, HC, 32, 32], BF16)
    nc.vector.memset(v32, 0.0)
    for c in range(HC):
        v32c = v32[:, c]
        diag = bass.AP(tensor=v32c.tensor, offset=v32c.offset,
                       ap=[v32c.ap[0], [33, 32]])
        nc.vector.tensor_scalar_add(out=diag, in0=diag, scalar1=v_sb32[:, c:c + 1])
    # mask bias row: (m - 1) * 1e9  ->  0 for kept, -1e9 for masked
    mask_f32 = singles.tile([1, TE], F32)
    nc.vector.tensor_copy(out=mask_f32, in_=mask_u8)
    mask_bias = singles.tile([1, TE], BF16)
    nc.vector.tensor_scalar(out=mask_bias, in0=mask_f32, scalar1=1e9, scalar2=-1e9,
                            op0=mult, op1=add)

    # ---------------- transposes: enc_T [h, s], dec_T [h, t] ----------------
    enc_T = singles.tile([P, HC, TE], F32)  # [p, hb, s] = enc[s, hb*128+p]
    for sb in range(2):
        for hb in range(HC):
            pt = psum.tile([P, P], F32, tag="tr")
            nc.tensor.transpose(pt, enc_sb[:, sb, hb * P:(hb + 1) * P], ident)
            nc.vector.tensor_copy(out=enc_T[:, hb, sb * P:(sb + 1) * P], in_=pt)
    dec_T = singles.tile([P, HC, TD], F32)
    for hb in range(HC):
        pt = psum.tile([P, P], F32, tag="tr")
        nc.tensor.transpose(pt, dec_sb[:, hb * P:(hb + 1) * P], ident)
        nc.vector.tensor_copy(out=dec_T[:, hb, :], in_=pt)

    # ---------------- projections: ew_T [k, s], dw_T [k, t] ----------------
    ew_T = singles.tile([P, HC, TE], F32)  # [p, kc, s] = ew[s, kc*128+p]
    for kc in range(HC):
        pew = psum.tile([P, TE], F32, tag="proj")
        for hb in range(HC):
            nc.tensor.matmul(pew, lhsT=w1_sb[:, hb, kc * P:(kc + 1) * P],
                             rhs=enc_T[:, hb, :], start=(hb == 0), stop=(hb == HC - 1))
        nc.vector.tensor_copy(out=ew_T[:, kc, :], in_=pew)
    dw_T = singles.tile([P, HC, TD], F32)
    for kc in range(HC):
        pdw = psum.tile([P, TD], F32, tag="proj")
        for hb in range(HC):
            nc.tensor.matmul(pdw, lhsT=w2_sb[:, hb, kc * P:(kc + 1) * P],
                             rhs=dec_T[:, hb, :], start=(hb == 0), stop=(hb == HC - 1))
        nc.vector.tensor_copy(out=dw_T[:, kc, :], in_=pdw)

    # ---------------- score accumulation ----------------
    score_ps = score_pool.tile([P, TE], F32)  # [t, s]
    # mask bias broadcast into every row t: ones[1,128].T @ mask_bias[1,256]
    nc.tensor.matmul(score_ps, lhsT=ones_row, rhs=mask_bias, start=True, stop=False,
                     skip_group_check=True)

    for g in range(NG):
        X = xpool.tile([P, TG * HC * TE], BF16)
        for tl in range(TG):
            t = g * TG + tl
            for c in range(HC):
                j = tl * HC + c
                nc.vector.tensor_scalar(out=X[:, j * TE:(j + 1) * TE],
                                        in0=ew_T[:, c, :],
                                        scalar1=dw_T[:, c, t:t + 1],
                                        scalar2=None, op0=add)
        Y = ypool.tile([P, TG * HC * TE], BF16)
        nc.scalar.activation(out=Y, in_=X, func=mybir.ActivationFunctionType.Tanh)
        for tl in range(TG):
            t = g * TG + tl
            strip = (t // 32) * 32
            jj = t % 32
            for c in range(HC):
                j = tl * HC + c
                last = (g == NG - 1) and (tl == TG - 1) and (c == HC - 1)
                nc.tensor.matmul(score_ps[strip:strip + 32, :],
                                 lhsT=v32[:, c, jj, :],
                                 rhs=Y[:, j * TE:(j + 1) * TE], start=False, stop=last,
                                 skip_group_check=True, tile_position=(0, strip))

    # ---------------- softmax over s (no max-subtraction needed:
    # |score| <= sum|V| ~ 16, exp fits easily in fp32) ----------------
    p_sb = singles.tile([P, TE], F32)
    nc.scalar.activation(out=p_sb, in_=score_ps, func=mybir.ActivationFunctionType.Exp)
    den = singles.tile([P, 1], F32)
    nc.vector.tensor_reduce(out=den, in_=p_sb, axis=mybir.AxisListType.X, op=add)
    rec = singles.tile([P, 1], F32)
    nc.vector.reciprocal(out=rec, in_=den)
    attn_sb = singles.tile([P, TE], F32)
    nc.vector.tensor_scalar(out=attn_sb, in0=p_sb, scalar1=rec[:, 0:1], scalar2=None,
                            op0=mult)
    nc.sync.dma_start(out=attn_out, in_=attn_sb)

    # ---------------- context: ctx[t, h] = sum_s attn[t, s] enc[s, h] ----------------
    attn_T = singles.tile([P, 2, P], F32)  # [s, sb, t]
    for sb in range(2):
        pt = psum.tile([P, P], F32, tag="tr")
        nc.tensor.transpose(pt, attn_sb[:, sb * P:(sb + 1) * P], ident)
        nc.vector.tensor_copy(out=attn_T[:, sb, :], in_=pt)
    ctx_ps = psum.tile([P, H], F32, tag="ctx")
    for sb in range(2):
        nc.tensor.matmul(ctx_ps, lhsT=attn_T[:, sb, :], rhs=enc_sb[:, sb, :],
                         start=(sb == 0), stop=(sb == 1))
    ctx_sb = singles.tile([P, H], F32)
    nc.vector.tensor_copy(out=ctx_sb, in_=ctx_ps)
    nc.sync.dma_start(out=ctx_out, in_=ctx_sb)


def build():
    nc = bacc.Bacc("TRN2", target_bir_lowering=False, debug=False, num_devices=B)
    enc = nc.dram_tensor("enc", (TE, H), F32, kind="ExternalInput").ap()
    dec = nc.dram_tensor("dec", (TD, H), F32, kind="ExternalInput").ap()
    mask = nc.dram_tensor("mask", (TE,), mybir.dt.uint8, kind="ExternalInput").ap()
    w1 = nc.dram_tensor("w1", (H, H), F32, kind="ExternalInput").ap()
    w2 = nc.dram_tensor("w2", (H, H), F32, kind="ExternalInput").ap()
    v = nc.dram_tensor("v", (H, 1), F32, kind="ExternalInput").ap()
    ctx_out = nc.dram_tensor("ctx_out", (TD, H), F32, kind="ExternalOutput").ap()
    attn_out = nc.dram_tensor("attn_out", (TD, TE), F32, kind="ExternalOutput").ap()
    with tile.TileContext(nc) as tc:
        _attention_kernel(tc, enc, dec, mask, w1, w2, v, ctx_out, attn_out)
    nc.compile()
    return nc


_NC_CACHE = None


def _get_nc():
    global _NC_CACHE
    if _NC_CACHE is None:
        _NC_CACHE = build()
    return _NC_CACHE


def make_in_maps(encoder_outputs, decoder_outputs, encoder_mask, W1, W2, V):
    enc = np.ascontiguousarray(np.asarray(encoder_outputs, dtype=np.float32))
    dec = np.ascontiguousarray(np.asarray(decoder_outputs, dtype=np.float32))
    msk = np.ascontiguousarray(np.asarray(encoder_mask).astype(np.uint8))
    w1 = np.ascontiguousarray(np.asarray(W1, dtype=np.float32))
    w2 = np.ascontiguousarray(np.asarray(W2, dtype=np.float32))
    v = np.ascontiguousarray(np.asarray(V, dtype=np.float32))
    return [
        {"enc": enc[b], "dec": dec[b], "mask": msk[b], "w1": w1, "w2": w2, "v": v}
        for b in range(B)
    ]


def kernel(encoder_outputs, decoder_outputs, encoder_mask, W1, W2, V, **run_kwargs):
    nc = _get_nc()
    in_maps = make_in_maps(encoder_outputs, decoder_outputs, encoder_mask, W1, W2, V)
    res = bass_utils.run_bass_kernel_spmd(nc, in_maps, core_ids=list(range(B)),
                                          **run_kwargs)
    ctx = np.stack([res.results[b]["ctx_out"] for b in range(B)])
    attn = np.stack([res.results[b]["attn_out"] for b in range(B)])
    return ctx, attn


# revision 11
# speedup vs baseline: 1.1274x; 1.1274x over previous
"""Bahdanau additive attention kernel for Trainium2 (Bass/Tile).

Shapes (hardcoded from the problem spec):
  encoder_outputs [8, 256, 512] f32, decoder_outputs [8, 128, 512] f32,
  encoder_mask    [8, 256] bool,  W1/W2 [512, 512] f32,  V [512, 1] f32.

Sharding: data-parallel over batch B=8 across the 8 NeuronCores; the
W1/W2/V weights are replicated.  Each core computes one batch element:
  ew = enc @ W1; dw = dec @ W2
  score[t,s] = sum_h V[h] * tanh(ew[s,h] + dw[t,h]) + (m[s]-1)*1e9
  attn = softmax_s(score); ctx = attn @ enc

Engine split per core (the Td*Te*H = 16.7M element inner part):
  - DVE: builds X[h, (t,c,s)] = ew_T + dw_T[:, t] via per-partition-scalar
    adds (2x mode, 512 instrs of [128, 256])
  - ACT: tanh on [128, 8192] tiles (16 instrs -> amortizes the 222-cycle
    SBUF access bubble; ACT is the ~110us/core roofline of this problem)
  - PE : score accumulation via M=1 matmuls with V as stationary operand,
    mask add as a K=1 broadcast matmul, plus projections/transposes/context
"""

from contextlib import ExitStack

import numpy as np

import concourse.bass as bass
import concourse.tile as tile
from concourse import bacc, bass_utils, mybir
from concourse._compat import with_exitstack
from concourse.masks import make_identity

B, TD, TE, H = 8, 128, 256, 512
P = 128
HC = H // P  # 4 h-chunks
TG = 8  # t's per ACT group
NG = TD // TG
F32 = mybir.dt.float32
BF16 = mybir.dt.bfloat16
F16 = mybir.dt.float16


@with_exitstack
def _attention_kernel(ctx: ExitStack, tc: tile.TileContext, enc, dec, mask, w1, w2, v,
                      ctx_out, attn_out):
    nc = tc.nc
    add = mybir.AluOpType.add
    mult = mybir.AluOpType.mult

    singles = ctx.enter_context(tc.tile_pool(name="singles", bufs=1))
    xpool = ctx.enter_context(tc.tile_pool(name="xpool", bufs=2))
    ypool = ctx.enter_context(tc.tile_pool(name="ypool", bufs=2))
    psum = ctx.enter_context(tc.tile_pool(name="psum", bufs=2, space="PSUM"))
    score_pool = ctx.enter_context(tc.tile_pool(name="score", bufs=1, space="PSUM"))

    # ---------------- loads ----------------
    w1_sb = singles.tile([P, HC, H], F32)  # [p, hb, k] = W1[hb*128+p, k]
    nc.sync.dma_start(out=w1_sb, in_=w1.rearrange("(hb p) k -> p hb k", p=P))
    w2_sb = singles.tile([P, HC, H], F32)
    nc.sync.dma_start(out=w2_sb, in_=w2.rearrange("(hb p) k -> p hb k", p=P))
    enc_sb = singles.tile([P, 2, H], F32)  # [p, sb, h] = enc[sb*128+p, h]
    nc.sync.dma_start(out=enc_sb, in_=enc.rearrange("(sb p) h -> p sb h", p=P))
    dec_sb = singles.tile([P, H], F32)
    nc.sync.dma_start(out=dec_sb, in_=dec)
    v_sb32 = singles.tile([P, HC], F32)  # [p, c] = V[c*128+p, 0]
    nc.sync.dma_start(out=v_sb32, in_=v.rearrange("(c p) o -> p (c o)", p=P))
    mask_u8 = singles.tile([1, TE], mybir.dt.uint8)
    nc.sync.dma_start(out=mask_u8, in_=mask.rearrange("(o s) -> o s", o=1))

    # ---------------- constants ----------------
    ident = singles.tile([P, P], F32)
    make_identity(nc, ident)
    ones_row = singles.tile([1, P], F16)
    nc.vector.memset(ones_row, 1.0)
    v_sb = singles.tile([P, HC], F16)
    nc.vector.tensor_copy(out=v_sb, in_=v_sb32)
    # V32[:, c, j, :] is a [128, 32] one-hot-column stationary operand:
    # column j holds V chunk c, all other columns zero.  A matmul with it
    # accumulates V_c . Y into score row (strip_base + j) while adding zero
    # to the other 31 rows of the strip (PSUM out must be 32-row aligned).
    v32 = singles.tile([P, HC, 32, 32], F16)
    nc.vector.memset(v32, 0.0)
    for c in range(HC):
        v32c = v32[:, c]
        diag = bass.AP(tensor=v32c.tensor, offset=v32c.offset,
                       ap=[v32c.ap[0], [33, 32]])
        nc.vector.tensor_scalar_add(out=diag, in0=diag, scalar1=v_sb32[:, c:c + 1])
    # mask bias row: (m - 1) * 60000 -> 0 for kept, -60000 for masked.
    # (-1e9 like the reference would overflow fp16; any bias <= -1e3 gives
    # exp(score + bias) == 0.0 exactly in fp32, matching the reference.)
    mask_f32 = singles.tile([1, TE], F32)
    nc.vector.tensor_copy(out=mask_f32, in_=mask_u8)
    mask_bias = singles.tile([1, TE], F16)
    nc.vector.tensor_scalar(out=mask_bias, in0=mask_f32, scalar1=6e4, scalar2=-6e4,
                            op0=mult, op1=add)

    # ---------------- transposes: enc_T [h, s], dec_T [h, t] ----------------
    enc_T = singles.tile([P, HC, TE], F32)  # [p, hb, s] = enc[s, hb*128+p]
    for sb in range(2):
        for hb in range(HC):
            pt = psum.tile([P, P], F32, tag="tr")
            nc.tensor.transpose(pt, enc_sb[:, sb, hb * P:(hb + 1) * P], ident)
            nc.vector.tensor_copy(out=enc_T[:, hb, sb * P:(sb + 1) * P], in_=pt)
    dec_T = singles.tile([P, HC, TD], F32)
    for hb in range(HC):
        pt = psum.tile([P, P], F32, tag="tr")
        nc.tensor.transpose(pt, dec_sb[:, hb * P:(hb + 1) * P], ident)
        nc.vector.tensor_copy(out=dec_T[:, hb, :], in_=pt)

    # ---------------- projections: ew_T [k, s], dw_T [k, t] ----------------
    ew_T = singles.tile([P, HC, TE], F16)  # [p, kc, s] = ew[s, kc*128+p]
    for kc in range(HC):
        pew = psum.tile([P, TE], F32, tag="proj")
        for hb in range(HC):
            nc.tensor.matmul(pew, lhsT=w1_sb[:, hb, kc * P:(kc + 1) * P],
                             rhs=enc_T[:, hb, :], start=(hb == 0), stop=(hb == HC - 1))
        nc.vector.tensor_copy(out=ew_T[:, kc, :], in_=pew)
    dw_T = singles.tile([P, HC, TD], F32)
    for kc in range(HC):
        pdw = psum.tile([P, TD], F32, tag="proj")
        for hb in range(HC):
            nc.tensor.matmul(pdw, lhsT=w2_sb[:, hb, kc * P:(kc + 1) * P],
                             rhs=dec_T[:, hb, :], start=(hb == 0), stop=(hb == HC - 1))
        nc.vector.tensor_copy(out=dw_T[:, kc, :], in_=pdw)

    # ---------------- score accumulation ----------------
    score_ps = score_pool.tile([P, TE], F32)  # [t, s]
    # mask bias broadcast into every row t: ones[1,128].T @ mask_bias[1,256]
    nc.tensor.matmul(score_ps, lhsT=ones_row, rhs=mask_bias, start=True, stop=False,
                     skip_group_check=True)

    for g in range(NG):
        X = xpool.tile([P, TG * HC * TE], F16)
        for tl in range(TG):
            t = g * TG + tl
            for c in range(HC):
                j = tl * HC + c
                nc.vector.tensor_scalar(out=X[:, j * TE:(j + 1) * TE],
                                        in0=ew_T[:, c, :],
                                        scalar1=dw_T[:, c, t:t + 1],
                                        scalar2=None, op0=add)
        Y = ypool.tile([P, TG * HC * TE], F16)
        nc.scalar.activation(out=Y, in_=X, func=mybir.ActivationFunctionType.Tanh)
        for tl in range(TG):
            t = g * TG + tl
            strip = (t // 32) * 32
            jj = t % 32
            for c in range(HC):
                j = tl * HC + c
                last = (g == NG - 1) and (tl == TG - 1) and (c == HC - 1)
                nc.tensor.matmul(score_ps[strip:strip + 32, :],
                                 lhsT=v32[:, c, jj, :],
                                 rhs=Y[:, j * TE:(j + 1) * TE], start=False, stop=last,
                                 skip_group_check=True, tile_position=(0, strip))

    # ---------------- softmax over s (no max-subtraction needed:
    # |score| <= sum|V| ~ 16, exp fits easily in fp32) ----------------
    p_sb = singles.tile([P, TE], F32)
    nc.scalar.activation(out=p_sb, in_=score_ps, func=mybir.ActivationFunctionType.Exp)
    den = singles.tile([P, 1], F32)
    nc.vector.tensor_reduce(out=den, in_=p_sb, axis=mybir.AxisListType.X, op=add)
    rec = singles.tile([P, 1], F32)
    nc.vector.reciprocal(out=rec, in_=den)
    attn_sb = singles.tile([P, TE], F32)
    nc.vector.tensor_scalar(out=attn_sb, in0=p_sb, scalar1=rec[:, 0:1], scalar2=None,
                            op0=mult)
    nc.sync.dma_start(out=attn_out, in_=attn_sb)

    # ---------------- context: ctx[t, h] = sum_s attn[t, s] enc[s, h] ----------------
    attn_T = singles.tile([P, 2, P], F32)  # [s, sb, t]
    for sb in range(2):
        pt = psum.tile([P, P], F32, tag="tr")
        nc.tensor.transpose(pt, attn_sb[:, sb * P:(sb + 1) * P], ident)
        nc.vector.tensor_copy(out=attn_T[:, sb, :], in_=pt)
    ctx_ps = psum.tile([P, H], F32, tag="ctx")
    for sb in range(2):
        nc.tensor.matmul(ctx_ps, lhsT=attn_T[:, sb, :], rhs=enc_sb[:, sb, :],
                         start=(sb == 0), stop=(sb == 1))
    ctx_sb = singles.tile([P, H], F32)
    nc.vector.tensor_copy(out=ctx_sb, in_=ctx_ps)
    nc.sync.dma_start(out=ctx_out, in_=ctx_sb)


def build():
    nc = bacc.Bacc("TRN2", target_bir_lowering=False, debug=False, num_devices=B)
    enc = nc.dram_tensor("enc", (TE, H), F32, kind="ExternalInput").ap()
    dec = nc.dram_tensor("dec", (TD, H), F32, kind="ExternalInput").ap()
    mask = nc.dram_tensor("mask", (TE,), mybir.dt.uint8, kind="ExternalInput").ap()
    w1 = nc.dram_tensor("w1", (H, H), F32, kind="ExternalInput").ap()
    w2 = nc.dram_tensor("w2", (H, H), F32, kind="ExternalInput").ap()
    v = nc.dram_tensor("v", (H, 1), F32, kind="ExternalInput").ap()
    ctx_out = nc.dram_tensor("ctx_out", (TD, H), F32, kind="ExternalOutput").ap()
    attn_out = nc.dram_tensor("attn_out", (TD, TE), F32, kind="ExternalOutput").ap()
    with tile.TileContext(nc) as tc:
        _attention_kernel(tc, enc, dec, mask, w1, w2, v, ctx_out, attn_out)
    nc.compile()
    return nc


_NC_CACHE = None


def _get_nc():
    global _NC_CACHE
    if _NC_CACHE is None:
        _NC_CACHE = build()
    return _NC_CACHE


def make_in_maps(encoder_outputs, decoder_outputs, encoder_mask, W1, W2, V):
    enc = np.ascontiguousarray(np.asarray(encoder_outputs, dtype=np.float32))
    dec = np.ascontiguousarray(np.asarray(decoder_outputs, dtype=np.float32))
    msk = np.ascontiguousarray(np.asarray(encoder_mask).astype(np.uint8))
    w1 = np.ascontiguousarray(np.asarray(W1, dtype=np.float32))
    w2 = np.ascontiguousarray(np.asarray(W2, dtype=np.float32))
    v = np.ascontiguousarray(np.asarray(V, dtype=np.float32))
    return [
        {"enc": enc[b], "dec": dec[b], "mask": msk[b], "w1": w1, "w2": w2, "v": v}
        for b in range(B)
    ]


def kernel(encoder_outputs, decoder_outputs, encoder_mask, W1, W2, V, **run_kwargs):
    nc = _get_nc()
    in_maps = make_in_maps(encoder_outputs, decoder_outputs, encoder_mask, W1, W2, V)
    res = bass_utils.run_bass_kernel_spmd(nc, in_maps, core_ids=list(range(B)),
                                          **run_kwargs)
    ctx = np.stack([res.results[b]["ctx_out"] for b in range(B)])
    attn = np.stack([res.results[b]["attn_out"] for b in range(B)])
    return ctx, attn


# revision 16
# speedup vs baseline: 1.1685x; 1.0364x over previous
"""Bahdanau additive attention kernel for Trainium2 (Bass/Tile).

Shapes (hardcoded from the problem spec):
  encoder_outputs [8, 256, 512] f32, decoder_outputs [8, 128, 512] f32,
  encoder_mask    [8, 256] bool,  W1/W2 [512, 512] f32,  V [512, 1] f32.

Sharding: data-parallel over batch B=8 across the 8 NeuronCores; the
W1/W2/V weights are replicated.  Each core computes one batch element:
  ew = enc @ W1; dw = dec @ W2
  score[t,s] = sum_h V[h] * tanh(ew[s,h] + dw[t,h]) + (m[s]-1)*1e9
  attn = softmax_s(score); ctx = attn @ enc

Engine split per core (the Td*Te*H = 16.7M element inner part):
  - DVE: builds X[h, (t,c,s)] = ew_T + dw_T[:, t] via per-partition-scalar
    adds (2x mode, 512 instrs of [128, 256])
  - ACT: tanh on [128, 8192] tiles (16 instrs -> amortizes the 222-cycle
    SBUF access bubble; ACT is the ~110us/core roofline of this problem)
  - PE : score accumulation via M=1 matmuls with V as stationary operand,
    mask add as a K=1 broadcast matmul, plus projections/transposes/context
"""

from contextlib import ExitStack

import numpy as np

import concourse.bass as bass
import concourse.tile as tile
from concourse import bacc, bass_utils, mybir
from concourse._compat import with_exitstack
from concourse.masks import make_identity

B, TD, TE, H = 8, 128, 256, 512
P = 128
HC = H // P  # 4 h-chunks
TG = 8  # t's per ACT group
NG = TD // TG
F32 = mybir.dt.float32
BF16 = mybir.dt.bfloat16
F16 = mybir.dt.float16


@with_exitstack
def _attention_kernel(ctx: ExitStack, tc: tile.TileContext, enc, dec, mask, w1, w2, v,
                      ctx_out, attn_out):
    nc = tc.nc
    add = mybir.AluOpType.add
    mult = mybir.AluOpType.mult

    singles = ctx.enter_context(tc.tile_pool(name="singles", bufs=1))
    xpool = ctx.enter_context(tc.tile_pool(name="xpool", bufs=2))
    ypool = ctx.enter_context(tc.tile_pool(name="ypool", bufs=2))
    psum = ctx.enter_context(tc.tile_pool(name="psum", bufs=3, space="PSUM"))
    score_pool = ctx.enter_context(tc.tile_pool(name="score", bufs=1, space="PSUM"))
    ctx_psum_pool = ctx.enter_context(tc.tile_pool(name="ctxp", bufs=1, space="PSUM"))

    # ---------------- loads ----------------
    # Issue order matters: dec/enc feed the PE transposes that unblock the
    # projections, so they go first; spread dispatch over several queues.
    dec_sb = singles.tile([P, H], F32)
    nc.sync.dma_start(out=dec_sb, in_=dec)
    enc_sb = singles.tile([P, 2, H], F32)  # [p, sb, h] = enc[sb*128+p, h]
    nc.gpsimd.dma_start(out=enc_sb, in_=enc.rearrange("(sb p) h -> p sb h", p=P))
    v_sb32 = singles.tile([P, HC], F32)  # [p, c] = V[c*128+p, 0]
    nc.gpsimd.dma_start(out=v_sb32, in_=v.rearrange("(c p) o -> p (c o)", p=P))
    mask_u8 = singles.tile([1, TE], mybir.dt.uint8)
    nc.gpsimd.dma_start(out=mask_u8, in_=mask.rearrange("(o s) -> o s", o=1))
    w2_sb = singles.tile([P, HC, H], F32)
    nc.scalar.dma_start(out=w2_sb, in_=w2.rearrange("(hb p) k -> p hb k", p=P))
    w1_sb = singles.tile([P, HC, H], F32)  # [p, hb, k] = W1[hb*128+p, k]
    nc.sync.dma_start(out=w1_sb, in_=w1.rearrange("(hb p) k -> p hb k", p=P))

    # ---------------- constants ----------------
    ident = singles.tile([P, P], F32)
    make_identity(nc, ident)
    ones_row = singles.tile([1, P], F16)
    nc.vector.memset(ones_row, 1.0)
    v_sb = singles.tile([P, HC], F16)
    nc.vector.tensor_copy(out=v_sb, in_=v_sb32)
    # V32[:, c, j, :] is a [128, 32] one-hot-column stationary operand:
    # column j holds V chunk c, all other columns zero.  A matmul with it
    # accumulates V_c . Y into score row (strip_base + j) while adding zero
    # to the other 31 rows of the strip (PSUM out must be 32-row aligned).
    v32 = singles.tile([P, HC, 32, 32], F16)
    nc.vector.memset(v32, 0.0)
    for c in range(HC):
        v32c = v32[:, c]
        diag = bass.AP(tensor=v32c.tensor, offset=v32c.offset,
                       ap=[v32c.ap[0], [33, 32]])
        nc.vector.tensor_scalar_add(out=diag, in0=diag, scalar1=v_sb32[:, c:c + 1])
    # mask bias row: (m - 1) * 60000 -> 0 for kept, -60000 for masked.
    # (-1e9 like the reference would overflow fp16; any bias <= -1e3 gives
    # exp(score + bias) == 0.0 exactly in fp32, matching the reference.)
    mask_f32 = singles.tile([1, TE], F32)
    nc.vector.tensor_copy(out=mask_f32, in_=mask_u8)
    mask_bias = singles.tile([1, TE], F16)
    nc.vector.tensor_scalar(out=mask_bias, in0=mask_f32, scalar1=6e4, scalar2=-6e4,
                            op0=mult, op1=add)

    # ---------------- transposes + projections ----------------
    # dec chain first: dec_T [h, t] then dw_T [k, t] (the X-build scalars),
    # then the enc chain: enc_T [h, s] and ew_T [k, s].
    dec_T = singles.tile([P, HC, TD], F32)
    for hb in range(HC):
        pt = psum.tile([P, P], F32, tag="tr")
        nc.tensor.transpose(pt, dec_sb[:, hb * P:(hb + 1) * P], ident)
        nc.vector.tensor_copy(out=dec_T[:, hb, :], in_=pt)
    dw_T = singles.tile([P, HC, TD], F32)
    for kc in range(HC):
        pdw = psum.tile([P, TD], F32, tag="proj")
        for hb in range(HC):
            nc.tensor.matmul(pdw, lhsT=w2_sb[:, hb, kc * P:(kc + 1) * P],
                             rhs=dec_T[:, hb, :], start=(hb == 0), stop=(hb == HC - 1))
        nc.vector.tensor_copy(out=dw_T[:, kc, :], in_=pdw)
    enc_T = singles.tile([P, HC, TE], F32)  # [p, hb, s] = enc[s, hb*128+p]
    for sb in range(2):
        for hb in range(HC):
            pt = psum.tile([P, P], F32, tag="tr")
            nc.tensor.transpose(pt, enc_sb[:, sb, hb * P:(hb + 1) * P], ident)
            nc.vector.tensor_copy(out=enc_T[:, hb, sb * P:(sb + 1) * P], in_=pt)
    ew_T = singles.tile([P, HC, TE], F16)  # [p, kc, s] = ew[s, kc*128+p]
    for kc in range(HC):
        pew = psum.tile([P, TE], F32, tag="proj")
        for hb in range(HC):
            nc.tensor.matmul(pew, lhsT=w1_sb[:, hb, kc * P:(kc + 1) * P],
                             rhs=enc_T[:, hb, :], start=(hb == 0), stop=(hb == HC - 1))
        nc.vector.tensor_copy(out=ew_T[:, kc, :], in_=pew)

    # ---------------- score accumulation ----------------
    score_ps = score_pool.tile([P, TE], F32)  # [t, s]
    # mask bias broadcast into every row t: ones[1,128].T @ mask_bias[1,256]
    nc.tensor.matmul(score_ps, lhsT=ones_row, rhs=mask_bias, start=True, stop=False,
                     skip_group_check=True)

    # Group-size schedule: small first groups let the ACT stream start as
    # soon as possible (less exposed X-build latency); small last groups
    # shorten the final tanh->V-matmul burst before the softmax.
    group_sizes = [2, 6] + [TG] * 14 + [4, 4]
    assert sum(group_sizes) == TD
    t0g = 0
    for gi, tg in enumerate(group_sizes):
        X = xpool.tile([P, TG * HC * TE], F16)
        for tl in range(tg):
            t = t0g + tl
            for c in range(HC):
                j = tl * HC + c
                nc.vector.tensor_scalar(out=X[:, j * TE:(j + 1) * TE],
                                        in0=ew_T[:, c, :],
                                        scalar1=dw_T[:, c, t:t + 1],
                                        scalar2=None, op0=add)
        Y = ypool.tile([P, TG * HC * TE], F16)
        nc.scalar.activation(out=Y[:, :tg * HC * TE], in_=X[:, :tg * HC * TE],
                             func=mybir.ActivationFunctionType.Tanh)
        for tl in range(tg):
            t = t0g + tl
            strip = (t // 32) * 32
            jj = t % 32
            for c in range(HC):
                j = tl * HC + c
                last = (t == TD - 1) and (c == HC - 1)
                nc.tensor.matmul(score_ps[strip:strip + 32, :],
                                 lhsT=v32[:, c, jj, :],
                                 rhs=Y[:, j * TE:(j + 1) * TE], start=False, stop=last,
                                 skip_group_check=True, tile_position=(0, strip))
        t0g += tg

    # ---------------- softmax over s (no max-subtraction needed:
    # |score| <= sum|V| ~ 16, exp fits easily in fp32) ----------------
    p_sb = singles.tile([P, TE], F32)
    nc.scalar.activation(out=p_sb, in_=score_ps, func=mybir.ActivationFunctionType.Exp)
    den = singles.tile([P, 1], F32)
    nc.vector.tensor_reduce(out=den, in_=p_sb, axis=mybir.AxisListType.X, op=add)
    rec = singles.tile([P, 1], F32)
    nc.vector.reciprocal(out=rec, in_=den)
    attn_sb = singles.tile([P, TE], F32)
    nc.vector.tensor_scalar(out=attn_sb, in0=p_sb, scalar1=rec[:, 0:1], scalar2=None,
                            op0=mult)
    nc.sync.dma_start(out=attn_out, in_=attn_sb)

    # ---------------- context: ctx[t, h] = sum_s attn[t, s] enc[s, h] ----------------
    attn_T = singles.tile([P, 2, P], F32)  # [s, sb, t]
    for sb in range(2):
        pt = psum.tile([P, P], F32, tag="tr")
        nc.tensor.transpose(pt, attn_sb[:, sb * P:(sb + 1) * P], ident)
        nc.vector.tensor_copy(out=attn_T[:, sb, :], in_=pt)
    ctx_ps = ctx_psum_pool.tile([P, H], F32)
    for sb in range(2):
        nc.tensor.matmul(ctx_ps, lhsT=attn_T[:, sb, :], rhs=enc_sb[:, sb, :],
                         start=(sb == 0), stop=(sb == 1))
    ctx_sb = singles.tile([P, H], F32)
    nc.vector.tensor_copy(out=ctx_sb, in_=ctx_ps)
    nc.sync.dma_start(out=ctx_out, in_=ctx_sb)


def build():
    nc = bacc.Bacc("TRN2", target_bir_lowering=False, debug=False, num_devices=B)
    enc = nc.dram_tensor("enc", (TE, H), F32, kind="ExternalInput").ap()
    dec = nc.dram_tensor("dec", (TD, H), F32, kind="ExternalInput").ap()
    mask = nc.dram_tensor("mask", (TE,), mybir.dt.uint8, kind="ExternalInput").ap()
    w1 = nc.dram_tensor("w1", (H, H), F32, kind="ExternalInput").ap()
    w2 = nc.dram_tensor("w2", (H, H), F32, kind="ExternalInput").ap()
    v = nc.dram_tensor("v", (H, 1), F32, kind="ExternalInput").ap()
    ctx_out = nc.dram_tensor("ctx_out", (TD, H), F32, kind="ExternalOutput").ap()
    attn_out = nc.dram_tensor("attn_out", (TD, TE), F32, kind="ExternalOutput").ap()
    with tile.TileContext(nc) as tc:
        _attention_kernel(tc, enc, dec, mask, w1, w2, v, ctx_out, attn_out)
    nc.compile()
    return nc


_NC_CACHE = None


def _get_nc():
    global _NC_CACHE
    if _NC_CACHE is None:
        _NC_CACHE = build()
    return _NC_CACHE


def make_in_maps(encoder_outputs, decoder_outputs, encoder_mask, W1, W2, V):
    enc = np.ascontiguousarray(np.asarray(encoder_outputs, dtype=np.float32))
    dec = np.ascontiguousarray(np.asarray(decoder_outputs, dtype=np.float32))
    msk = np.ascontiguousarray(np.asarray(encoder_mask).astype(np.uint8))
    w1 = np.ascontiguousarray(np.asarray(W1, dtype=np.float32))
    w2 = np.ascontiguousarray(np.asarray(W2, dtype=np.float32))
    v = np.ascontiguousarray(np.asarray(V, dtype=np.float32))
    return [
        {"enc": enc[b], "dec": dec[b], "mask": msk[b], "w1": w1, "w2": w2, "v": v}
        for b in range(B)
    ]


def kernel(encoder_outputs, decoder_outputs, encoder_mask, W1, W2, V, **run_kwargs):
    nc = _get_nc()
    in_maps = make_in_maps(encoder_outputs, decoder_outputs, encoder_mask, W1, W2, V)
    res = bass_utils.run_bass_kernel_spmd(nc, in_maps, core_ids=list(range(B)),
                                          **run_kwargs)
    ctx = np.stack([res.results[b]["ctx_out"] for b in range(B)])
    attn = np.stack([res.results[b]["attn_out"] for b in range(B)])
    return ctx, attn


# revision 17
# speedup vs baseline: 1.1841x; 1.0133x over previous
"""Bahdanau additive attention kernel for Trainium2 (Bass/Tile).

Shapes (hardcoded from the problem spec):
  encoder_outputs [8, 256, 512] f32, decoder_outputs [8, 128, 512] f32,
  encoder_mask    [8, 256] bool,  W1/W2 [512, 512] f32,  V [512, 1] f32.

Sharding: data-parallel over batch B=8 across the 8 NeuronCores; the
W1/W2/V weights are replicated.  Each core computes one batch element:
  ew = enc @ W1; dw = dec @ W2
  score[t,s] = sum_h V[h] * tanh(ew[s,h] + dw[t,h]) + (m[s]-1)*1e9
  attn = softmax_s(score); ctx = attn @ enc

Engine split per core (the Td*Te*H = 16.7M element inner part):
  - DVE: builds X[h, (t,c,s)] = ew_T + dw_T[:, t] via per-partition-scalar
    adds (2x mode, 512 instrs of [128, 256])
  - ACT: tanh on [128, 8192] tiles (16 instrs -> amortizes the 222-cycle
    SBUF access bubble; ACT is the ~110us/core roofline of this problem)
  - PE : score accumulation via M=1 matmuls with V as stationary operand,
    mask add as a K=1 broadcast matmul, plus projections/transposes/context
"""

from contextlib import ExitStack

import numpy as np

import concourse.bass as bass
import concourse.tile as tile
from concourse import bacc, bass_utils, mybir
from concourse._compat import with_exitstack
from concourse.masks import make_identity

B, TD, TE, H = 8, 128, 256, 512
P = 128
HC = H // P  # 4 h-chunks
TG = 8  # t's per ACT group
NG = TD // TG
F32 = mybir.dt.float32
BF16 = mybir.dt.bfloat16
F16 = mybir.dt.float16


@with_exitstack
def _attention_kernel(ctx: ExitStack, tc: tile.TileContext, enc, dec, mask, w1, w2, v,
                      ctx_out, attn_out):
    nc = tc.nc
    add = mybir.AluOpType.add
    mult = mybir.AluOpType.mult

    singles = ctx.enter_context(tc.tile_pool(name="singles", bufs=1))
    xpool = ctx.enter_context(tc.tile_pool(name="xpool", bufs=2))
    ypool = ctx.enter_context(tc.tile_pool(name="ypool", bufs=2))
    psum = ctx.enter_context(tc.tile_pool(name="psum", bufs=3, space="PSUM"))
    score_pool = ctx.enter_context(tc.tile_pool(name="score", bufs=1, space="PSUM"))
    ctx_psum_pool = ctx.enter_context(tc.tile_pool(name="ctxp", bufs=1, space="PSUM"))

    # ---------------- input-independent constants first ----------------
    # (make_identity ends with a GPSIMD drain that would otherwise serialize
    # behind any DMA already queued on the gpsimd engine)
    ident = singles.tile([P, P], F32)
    make_identity(nc, ident)
    ones_row = singles.tile([1, P], F16)
    nc.vector.memset(ones_row, 1.0)

    # ---------------- loads ----------------
    # Issue order matters: dec/enc feed the PE transposes that unblock the
    # projections, so they go first; spread dispatch over several queues.
    dec_sb = singles.tile([P, H], F32)
    nc.sync.dma_start(out=dec_sb, in_=dec)
    enc_sb = singles.tile([P, 2, H], F32)  # [p, sb, h] = enc[sb*128+p, h]
    nc.sync.dma_start(out=enc_sb, in_=enc.rearrange("(sb p) h -> p sb h", p=P))
    v_sb32 = singles.tile([P, HC], F32)  # [p, c] = V[c*128+p, 0]
    nc.gpsimd.dma_start(out=v_sb32, in_=v.rearrange("(c p) o -> p (c o)", p=P))
    mask_u8 = singles.tile([1, TE], mybir.dt.uint8)
    nc.gpsimd.dma_start(out=mask_u8, in_=mask.rearrange("(o s) -> o s", o=1))
    w2_sb = singles.tile([P, HC, H], F32)
    nc.scalar.dma_start(out=w2_sb, in_=w2.rearrange("(hb p) k -> p hb k", p=P))
    w1_sb = singles.tile([P, HC, H], F32)  # [p, hb, k] = W1[hb*128+p, k]
    nc.sync.dma_start(out=w1_sb, in_=w1.rearrange("(hb p) k -> p hb k", p=P))
    v_sb = singles.tile([P, HC], F16)
    nc.vector.tensor_copy(out=v_sb, in_=v_sb32)
    # V32[:, c, j, :] is a [128, 32] one-hot-column stationary operand:
    # column j holds V chunk c, all other columns zero.  A matmul with it
    # accumulates V_c . Y into score row (strip_base + j) while adding zero
    # to the other 31 rows of the strip (PSUM out must be 32-row aligned).
    v32 = singles.tile([P, HC, 32, 32], F16)
    nc.vector.memset(v32, 0.0)
    for c in range(HC):
        v32c = v32[:, c]
        diag = bass.AP(tensor=v32c.tensor, offset=v32c.offset,
                       ap=[v32c.ap[0], [33, 32]])
        nc.vector.tensor_scalar_add(out=diag, in0=diag, scalar1=v_sb32[:, c:c + 1])
    # mask bias row: (m - 1) * 60000 -> 0 for kept, -60000 for masked.
    # (-1e9 like the reference would overflow fp16; any bias <= -1e3 gives
    # exp(score + bias) == 0.0 exactly in fp32, matching the reference.)
    mask_f32 = singles.tile([1, TE], F32)
    nc.vector.tensor_copy(out=mask_f32, in_=mask_u8)
    mask_bias = singles.tile([1, TE], F16)
    nc.vector.tensor_scalar(out=mask_bias, in0=mask_f32, scalar1=6e4, scalar2=-6e4,
                            op0=mult, op1=add)

    # ---------------- transposes + projections ----------------
    # dec chain first: dec_T [h, t] then dw_T [k, t] (the X-build scalars),
    # then the enc chain: enc_T [h, s] and ew_T [k, s].
    dec_T = singles.tile([P, HC, TD], F32)
    for hb in range(HC):
        pt = psum.tile([P, P], F32, tag="tr")
        nc.tensor.transpose(pt, dec_sb[:, hb * P:(hb + 1) * P], ident)
        nc.vector.tensor_copy(out=dec_T[:, hb, :], in_=pt)
    dw_T = singles.tile([P, HC, TD], F32)
    for kc in range(HC):
        pdw = psum.tile([P, TD], F32, tag="proj")
        for hb in range(HC):
            nc.tensor.matmul(pdw, lhsT=w2_sb[:, hb, kc * P:(kc + 1) * P],
                             rhs=dec_T[:, hb, :], start=(hb == 0), stop=(hb == HC - 1))
        nc.vector.tensor_copy(out=dw_T[:, kc, :], in_=pdw)
    enc_T = singles.tile([P, HC, TE], F32)  # [p, hb, s] = enc[s, hb*128+p]
    for sb in range(2):
        for hb in range(HC):
            pt = psum.tile([P, P], F32, tag="tr")
            nc.tensor.transpose(pt, enc_sb[:, sb, hb * P:(hb + 1) * P], ident)
            nc.vector.tensor_copy(out=enc_T[:, hb, sb * P:(sb + 1) * P], in_=pt)
    ew_T = singles.tile([P, HC, TE], F16)  # [p, kc, s] = ew[s, kc*128+p]
    for kc in range(HC):
        pew = psum.tile([P, TE], F32, tag="proj")
        for hb in range(HC):
            nc.tensor.matmul(pew, lhsT=w1_sb[:, hb, kc * P:(kc + 1) * P],
                             rhs=enc_T[:, hb, :], start=(hb == 0), stop=(hb == HC - 1))
        nc.vector.tensor_copy(out=ew_T[:, kc, :], in_=pew)

    # ---------------- score accumulation ----------------
    score_ps = score_pool.tile([P, TE], F32)  # [t, s]
    # mask bias broadcast into every row t: ones[1,128].T @ mask_bias[1,256]
    nc.tensor.matmul(score_ps, lhsT=ones_row, rhs=mask_bias, start=True, stop=False,
                     skip_group_check=True)

    # Group-size schedule: small first groups let the ACT stream start as
    # soon as possible (less exposed X-build latency); small last groups
    # shorten the final tanh->V-matmul burst before the softmax.
    group_sizes = [2, 6] + [TG] * 14 + [4, 4]
    assert sum(group_sizes) == TD
    t0g = 0
    for gi, tg in enumerate(group_sizes):
        X = xpool.tile([P, TG * HC * TE], F16)
        for tl in range(tg):
            t = t0g + tl
            for c in range(HC):
                j = tl * HC + c
                nc.vector.tensor_scalar(out=X[:, j * TE:(j + 1) * TE],
                                        in0=ew_T[:, c, :],
                                        scalar1=dw_T[:, c, t:t + 1],
                                        scalar2=None, op0=add)
        Y = ypool.tile([P, TG * HC * TE], F16)
        nc.scalar.activation(out=Y[:, :tg * HC * TE], in_=X[:, :tg * HC * TE],
                             func=mybir.ActivationFunctionType.Tanh)
        for tl in range(tg):
            t = t0g + tl
            strip = (t // 32) * 32
            jj = t % 32
            for c in range(HC):
                j = tl * HC + c
                last = (t == TD - 1) and (c == HC - 1)
                nc.tensor.matmul(score_ps[strip:strip + 32, :],
                                 lhsT=v32[:, c, jj, :],
                                 rhs=Y[:, j * TE:(j + 1) * TE], start=False, stop=last,
                                 skip_group_check=True, tile_position=(0, strip))
        t0g += tg

    # ---------------- softmax over s (no max-subtraction needed:
    # |score| <= sum|V| ~ 16, exp fits easily in fp32) ----------------
    p_sb = singles.tile([P, TE], F32)
    nc.scalar.activation(out=p_sb, in_=score_ps, func=mybir.ActivationFunctionType.Exp)
    den = singles.tile([P, 1], F32)
    nc.vector.tensor_reduce(out=den, in_=p_sb, axis=mybir.AxisListType.X, op=add)
    rec = singles.tile([P, 1], F32)
    nc.vector.reciprocal(out=rec, in_=den)
    attn_sb = singles.tile([P, TE], F32)
    nc.vector.tensor_scalar(out=attn_sb, in0=p_sb, scalar1=rec[:, 0:1], scalar2=None,
                            op0=mult)
    nc.sync.dma_start(out=attn_out, in_=attn_sb)

    # ---------------- context: ctx[t, h] = sum_s attn[t, s] enc[s, h] ----------------
    attn_T = singles.tile([P, 2, P], F32)  # [s, sb, t]
    for sb in range(2):
        pt = psum.tile([P, P], F32, tag="tr")
        nc.tensor.transpose(pt, attn_sb[:, sb * P:(sb + 1) * P], ident)
        nc.vector.tensor_copy(out=attn_T[:, sb, :], in_=pt)
    ctx_ps = ctx_psum_pool.tile([P, H], F32)
    for sb in range(2):
        nc.tensor.matmul(ctx_ps, lhsT=attn_T[:, sb, :], rhs=enc_sb[:, sb, :],
                         start=(sb == 0), stop=(sb == 1))
    ctx_sb = singles.tile([P, H], F32)
    nc.vector.tensor_copy(out=ctx_sb, in_=ctx_ps)
    nc.sync.dma_start(out=ctx_out, in_=ctx_sb)


def build():
    nc = bacc.Bacc("TRN2", target_bir_lowering=False, debug=False, num_devices=B)
    enc = nc.dram_tensor("enc", (TE, H), F32, kind="ExternalInput").ap()
    dec = nc.dram_tensor("dec", (TD, H), F32, kind="ExternalInput").ap()
    mask = nc.dram_tensor("mask", (TE,), mybir.dt.uint8, kind="ExternalInput").ap()
    w1 = nc.dram_tensor("w1", (H, H), F32, kind="ExternalInput").ap()
    w2 = nc.dram_tensor("w2", (H, H), F32, kind="ExternalInput").ap()
    v = nc.dram_tensor("v", (H, 1), F32, kind="ExternalInput").ap()
    ctx_out = nc.dram_tensor("ctx_out", (TD, H), F32, kind="ExternalOutput").ap()
    attn_out = nc.dram_tensor("attn_out", (TD, TE), F32, kind="ExternalOutput").ap()
    with tile.TileContext(nc) as tc:
        _attention_kernel(tc, enc, dec, mask, w1, w2, v, ctx_out, attn_out)
    nc.compile()
    return nc


_NC_CACHE = None


def _get_nc():
    global _NC_CACHE
    if _NC_CACHE is None:
        _NC_CACHE = build()
    return _NC_CACHE


def make_in_maps(encoder_outputs, decoder_outputs, encoder_mask, W1, W2, V):
    enc = np.ascontiguousarray(np.asarray(encoder_outputs, dtype=np.float32))
    dec = np.ascontiguousarray(np.asarray(decoder_outputs, dtype=np.float32))
    msk = np.ascontiguousarray(np.asarray(encoder_mask).astype(np.uint8))
    w1 = np.ascontiguousarray(np.asarray(W1, dtype=np.float32))
    w2 = np.ascontiguousarray(np.asarray(W2, dtype=np.float32))
    v = np.ascontiguousarray(np.asarray(V, dtype=np.float32))
    return [
        {"enc": enc[b], "dec": dec[b], "mask": msk[b], "w1": w1, "w2": w2, "v": v}
        for b in range(B)
    ]


def kernel(encoder_outputs, decoder_outputs, encoder_mask, W1, W2, V, **run_kwargs):
    nc = _get_nc()
    in_maps = make_in_maps(encoder_outputs, decoder_outputs, encoder_mask, W1, W2, V)
    res = bass_utils.run_bass_kernel_spmd(nc, in_maps, core_ids=list(range(B)),
                                          **run_kwargs)
    ctx = np.stack([res.results[b]["ctx_out"] for b in range(B)])
    attn = np.stack([res.results[b]["attn_out"] for b in range(B)])
    return ctx, attn


# revision 18
# speedup vs baseline: 1.2005x; 1.0139x over previous
"""Bahdanau additive attention kernel for Trainium2 (Bass/Tile).

Shapes (hardcoded from the problem spec):
  encoder_outputs [8, 256, 512] f32, decoder_outputs [8, 128, 512] f32,
  encoder_mask    [8, 256] bool,  W1/W2 [512, 512] f32,  V [512, 1] f32.

Sharding: data-parallel over batch B=8 across the 8 NeuronCores; the
W1/W2/V weights are replicated.  Each core computes one batch element:
  ew = enc @ W1; dw = dec @ W2
  score[t,s] = sum_h V[h] * tanh(ew[s,h] + dw[t,h]) + (m[s]-1)*1e9
  attn = softmax_s(score); ctx = attn @ enc

Engine split per core (the Td*Te*H = 16.7M element inner part):
  - DVE: builds X[h, (t,c,s)] = ew_T + dw_T[:, t] via per-partition-scalar
    adds (2x mode, 512 instrs of [128, 256])
  - ACT: tanh on [128, 8192] tiles (16 instrs -> amortizes the 222-cycle
    SBUF access bubble; ACT is the ~110us/core roofline of this problem)
  - PE : score accumulation via M=1 matmuls with V as stationary operand,
    mask add as a K=1 broadcast matmul, plus projections/transposes/context
"""

from contextlib import ExitStack

import numpy as np

import concourse.bass as bass
import concourse.tile as tile
from concourse import bacc, bass_utils, mybir
from concourse._compat import with_exitstack
from concourse.masks import make_identity

B, TD, TE, H = 8, 128, 256, 512
P = 128
HC = H // P  # 4 h-chunks
TG = 8  # t's per ACT group
NG = TD // TG
F32 = mybir.dt.float32
BF16 = mybir.dt.bfloat16
F16 = mybir.dt.float16


@with_exitstack
def _attention_kernel(ctx: ExitStack, tc: tile.TileContext, enc, dec, mask, w1, w2, v,
                      ctx_out, attn_out):
    nc = tc.nc
    add = mybir.AluOpType.add
    mult = mybir.AluOpType.mult

    singles = ctx.enter_context(tc.tile_pool(name="singles", bufs=1))
    xpool = ctx.enter_context(tc.tile_pool(name="xpool", bufs=3))
    ypool = ctx.enter_context(tc.tile_pool(name="ypool", bufs=2))
    psum = ctx.enter_context(tc.tile_pool(name="psum", bufs=3, space="PSUM"))
    score_pool = ctx.enter_context(tc.tile_pool(name="score", bufs=1, space="PSUM"))
    ctx_psum_pool = ctx.enter_context(tc.tile_pool(name="ctxp", bufs=1, space="PSUM"))

    # ---------------- input-independent constants first ----------------
    # (make_identity ends with a GPSIMD drain that would otherwise serialize
    # behind any DMA already queued on the gpsimd engine)
    ident = singles.tile([P, P], F32)
    make_identity(nc, ident)
    ones_row = singles.tile([1, P], F16)
    nc.vector.memset(ones_row, 1.0)

    # ---------------- loads ----------------
    # Issue order matters: dec/enc feed the PE transposes that unblock the
    # projections, so they go first; spread dispatch over several queues.
    dec_sb = singles.tile([P, H], F32)
    nc.sync.dma_start(out=dec_sb, in_=dec)
    enc_sb = singles.tile([P, 2, H], F32)  # [p, sb, h] = enc[sb*128+p, h]
    nc.sync.dma_start(out=enc_sb, in_=enc.rearrange("(sb p) h -> p sb h", p=P))
    v_sb32 = singles.tile([P, HC], F32)  # [p, c] = V[c*128+p, 0]
    nc.gpsimd.dma_start(out=v_sb32, in_=v.rearrange("(c p) o -> p (c o)", p=P))
    mask_u8 = singles.tile([1, TE], mybir.dt.uint8)
    nc.gpsimd.dma_start(out=mask_u8, in_=mask.rearrange("(o s) -> o s", o=1))
    w2_sb = singles.tile([P, HC, H], F32)
    nc.scalar.dma_start(out=w2_sb, in_=w2.rearrange("(hb p) k -> p hb k", p=P))
    w1_sb = singles.tile([P, HC, H], F32)  # [p, hb, k] = W1[hb*128+p, k]
    nc.sync.dma_start(out=w1_sb, in_=w1.rearrange("(hb p) k -> p hb k", p=P))
    v_sb = singles.tile([P, HC], F16)
    nc.vector.tensor_copy(out=v_sb, in_=v_sb32)
    # V32[:, c, j, :] is a [128, 32] one-hot-column stationary operand:
    # column j holds V chunk c, all other columns zero.  A matmul with it
    # accumulates V_c . Y into score row (strip_base + j) while adding zero
    # to the other 31 rows of the strip (PSUM out must be 32-row aligned).
    v32 = singles.tile([P, HC, 32, 32], F16)
    nc.vector.memset(v32, 0.0)
    for c in range(HC):
        v32c = v32[:, c]
        diag = bass.AP(tensor=v32c.tensor, offset=v32c.offset,
                       ap=[v32c.ap[0], [33, 32]])
        nc.vector.tensor_scalar_add(out=diag, in0=diag, scalar1=v_sb32[:, c:c + 1])
    # mask bias row: (m - 1) * 60000 -> 0 for kept, -60000 for masked.
    # (-1e9 like the reference would overflow fp16; any bias <= -1e3 gives
    # exp(score + bias) == 0.0 exactly in fp32, matching the reference.)
    mask_f32 = singles.tile([1, TE], F32)
    nc.vector.tensor_copy(out=mask_f32, in_=mask_u8)
    mask_bias = singles.tile([1, TE], F16)
    nc.vector.tensor_scalar(out=mask_bias, in0=mask_f32, scalar1=6e4, scalar2=-6e4,
                            op0=mult, op1=add)

    # ---------------- PE warmup during the DMA wait ----------------
    # ~28 dummy identity matmuls keep the PE busy past the HAM activity
    # window so the projections below run at 2.4 GHz instead of 1.2 GHz.
    warm_ps = psum.tile([P, P], F32, tag="tr")
    for _ in range(28):
        nc.tensor.matmul(warm_ps, lhsT=ident, rhs=ident, start=True, stop=True)

    # ---------------- transposes + projections ----------------
    # dec chain first: dec_T [h, t] then dw_T [k, t] (the X-build scalars),
    # then the enc chain: enc_T [h, s] and ew_T [k, s].
    dec_T = singles.tile([P, HC, TD], F32)
    for hb in range(HC):
        pt = psum.tile([P, P], F32, tag="tr")
        nc.tensor.transpose(pt, dec_sb[:, hb * P:(hb + 1) * P], ident)
        nc.vector.tensor_copy(out=dec_T[:, hb, :], in_=pt)
    dw_T = singles.tile([P, HC, TD], F32)
    for kc in range(HC):
        pdw = psum.tile([P, TD], F32, tag="proj")
        for hb in range(HC):
            nc.tensor.matmul(pdw, lhsT=w2_sb[:, hb, kc * P:(kc + 1) * P],
                             rhs=dec_T[:, hb, :], start=(hb == 0), stop=(hb == HC - 1))
        nc.vector.tensor_copy(out=dw_T[:, kc, :], in_=pdw)
    enc_T = singles.tile([P, HC, TE], F32)  # [p, hb, s] = enc[s, hb*128+p]
    for sb in range(2):
        for hb in range(HC):
            pt = psum.tile([P, P], F32, tag="tr")
            nc.tensor.transpose(pt, enc_sb[:, sb, hb * P:(hb + 1) * P], ident)
            nc.vector.tensor_copy(out=enc_T[:, hb, sb * P:(sb + 1) * P], in_=pt)
    ew_T = singles.tile([P, HC, TE], F16)  # [p, kc, s] = ew[s, kc*128+p]
    for kc in range(HC):
        pew = psum.tile([P, TE], F32, tag="proj")
        for hb in range(HC):
            nc.tensor.matmul(pew, lhsT=w1_sb[:, hb, kc * P:(kc + 1) * P],
                             rhs=enc_T[:, hb, :], start=(hb == 0), stop=(hb == HC - 1))
        nc.vector.tensor_copy(out=ew_T[:, kc, :], in_=pew)

    # ---------------- score accumulation ----------------
    score_ps = score_pool.tile([P, TE], F32)  # [t, s]
    # mask bias broadcast into every row t: ones[1,128].T @ mask_bias[1,256]
    nc.tensor.matmul(score_ps, lhsT=ones_row, rhs=mask_bias, start=True, stop=False,
                     skip_group_check=True)

    # Group-size schedule: small first groups let the ACT stream start as
    # soon as possible (less exposed X-build latency); small last groups
    # shorten the final tanh->V-matmul burst before the softmax.
    group_sizes = [2, 6] + [TG] * 14 + [4, 2, 2]
    assert sum(group_sizes) == TD
    t0g = 0
    for gi, tg in enumerate(group_sizes):
        X = xpool.tile([P, TG * HC * TE], F16)
        for tl in range(tg):
            t = t0g + tl
            for c in range(HC):
                j = tl * HC + c
                nc.vector.tensor_scalar(out=X[:, j * TE:(j + 1) * TE],
                                        in0=ew_T[:, c, :],
                                        scalar1=dw_T[:, c, t:t + 1],
                                        scalar2=None, op0=add)
        Y = ypool.tile([P, TG * HC * TE], F16)
        nc.scalar.activation(out=Y[:, :tg * HC * TE], in_=X[:, :tg * HC * TE],
                             func=mybir.ActivationFunctionType.Tanh)
        for tl in range(tg):
            t = t0g + tl
            strip = (t // 32) * 32
            jj = t % 32
            for c in range(HC):
                j = tl * HC + c
                last = (t == TD - 1) and (c == HC - 1)
                nc.tensor.matmul(score_ps[strip:strip + 32, :],
                                 lhsT=v32[:, c, jj, :],
                                 rhs=Y[:, j * TE:(j + 1) * TE], start=False, stop=last,
                                 skip_group_check=True, tile_position=(0, strip))
        t0g += tg

    # ---------------- softmax over s (no max-subtraction needed:
    # |score| <= sum|V| ~ 16, exp fits easily in fp32) ----------------
    p_sb = singles.tile([P, TE], F32)
    nc.scalar.activation(out=p_sb, in_=score_ps, func=mybir.ActivationFunctionType.Exp)
    den = singles.tile([P, 1], F32)
    nc.vector.tensor_reduce(out=den, in_=p_sb, axis=mybir.AxisListType.X, op=add)
    rec = singles.tile([P, 1], F32)
    nc.vector.reciprocal(out=rec, in_=den)
    attn_sb = singles.tile([P, TE], F32)
    nc.vector.tensor_scalar(out=attn_sb, in0=p_sb, scalar1=rec[:, 0:1], scalar2=None,
                            op0=mult)
    nc.sync.dma_start(out=attn_out, in_=attn_sb)

    # ---------------- context: ctx[t, h] = sum_s attn[t, s] enc[s, h] ----------------
    attn_T = singles.tile([P, 2, P], F32)  # [s, sb, t]
    for sb in range(2):
        pt = psum.tile([P, P], F32, tag="tr")
        nc.tensor.transpose(pt, attn_sb[:, sb * P:(sb + 1) * P], ident)
        nc.vector.tensor_copy(out=attn_T[:, sb, :], in_=pt)
    ctx_ps = ctx_psum_pool.tile([P, H], F32)
    for sb in range(2):
        nc.tensor.matmul(ctx_ps, lhsT=attn_T[:, sb, :], rhs=enc_sb[:, sb, :],
                         start=(sb == 0), stop=(sb == 1))
    ctx_sb = singles.tile([P, H], F32)
    nc.vector.tensor_copy(out=ctx_sb, in_=ctx_ps)
    nc.sync.dma_start(out=ctx_out, in_=ctx_sb)


def build():
    nc = bacc.Bacc("TRN2", target_bir_lowering=False, debug=False, num_devices=B)
    enc = nc.dram_tensor("enc", (TE, H), F32, kind="ExternalInput").ap()
    dec = nc.dram_tensor("dec", (TD, H), F32, kind="ExternalInput").ap()
    mask = nc.dram_tensor("mask", (TE,), mybir.dt.uint8, kind="ExternalInput").ap()
    w1 = nc.dram_tensor("w1", (H, H), F32, kind="ExternalInput").ap()
    w2 = nc.dram_tensor("w2", (H, H), F32, kind="ExternalInput").ap()
    v = nc.dram_tensor("v", (H, 1), F32, kind="ExternalInput").ap()
    ctx_out = nc.dram_tensor("ctx_out", (TD, H), F32, kind="ExternalOutput").ap()
    attn_out = nc.dram_tensor("attn_out", (TD, TE), F32, kind="ExternalOutput").ap()
    with tile.TileContext(nc) as tc:
        _attention_kernel(tc, enc, dec, mask, w1, w2, v, ctx_out, attn_out)
    nc.compile()
    return nc


_NC_CACHE = None


def _get_nc():
    global _NC_CACHE
    if _NC_CACHE is None:
        _NC_CACHE = build()
    return _NC_CACHE


def make_in_maps(encoder_outputs, decoder_outputs, encoder_mask, W1, W2, V):
    enc = np.ascontiguousarray(np.asarray(encoder_outputs, dtype=np.float32))
    dec = np.ascontiguousarray(np.asarray(decoder_outputs, dtype=np.float32))
    msk = np.ascontiguousarray(np.asarray(encoder_mask).astype(np.uint8))
    w1 = np.ascontiguousarray(np.asarray(W1, dtype=np.float32))
    w2 = np.ascontiguousarray(np.asarray(W2, dtype=np.float32))
    v = np.ascontiguousarray(np.asarray(V, dtype=np.float32))
    return [
        {"enc": enc[b], "dec": dec[b], "mask": msk[b], "w1": w1, "w2": w2, "v": v}
        for b in range(B)
    ]


def kernel(encoder_outputs, decoder_outputs, encoder_mask, W1, W2, V, **run_kwargs):
    nc = _get_nc()
    in_maps = make_in_maps(encoder_outputs, decoder_outputs, encoder_mask, W1, W2, V)
    res = bass_utils.run_bass_kernel_spmd(nc, in_maps, core_ids=list(range(B)),
                                          **run_kwargs)
    ctx = np.stack([res.results[b]["ctx_out"] for b in range(B)])
    attn = np.stack([res.results[b]["attn_out"] for b in range(B)])
    return ctx, attn
